# revision 24
# baseline (speedup 1.0000x reference)
"""Trainium2 Bass kernel for nn_MoELayer_90202903150800 (MoE, 8 experts, top-2).

Sharding (8 NeuronCores): expert-parallel, one full expert per core, with
SPARSE routing computed on-device:

  Phase R (router): stream x, PE-transpose 128x128 chunks, plain-fp32 logits
    (exact top-2 selection), softmax-free renormalized combine weights
    (w = e_i / (m1 + m2) over top-2 exponentials).  A strict-triangular-matmul
    prefix sum compacts the tokens routed to this core's expert: each selected
    token's slot index s is scattered (indirect DMA) into idx[s] = token id,
    w[s] = combine weight.  Unselected tokens scatter out-of-bounds and are
    dropped.  Top-2 membership counts accumulate for the cv^2 aux loss.
  Phase G (gather): indirect-DMA gather the <=1152 routed token rows of x,
    PE-transpose into x_gT for matmul use.
  Phase M (MLP): stream W1/W2 in 8 F-chunks of 512 (weights pass over SBUF
    exactly once), computing h = gelu(x_g @ W1 + b1) and accumulating
    y = h @ W2 in SBUF, all in fp32r (full-rate fp32 on the PE).
  Finalize: out_partial[token] = (y + b2) * w, scattered back to token order
    in a zero-initialized [4096, 1024] buffer; single ReduceScatter over the
    8 cores sums the 8 expert partials and shards tokens 8 ways; host
    concatenates the shards.  cv^2 is computed on-device from counts.

SPMD: the same program runs on all cores; per-core behaviour differs only
via in_maps data (own expert's weights; Wr columns rolled so the own expert
is column 0 — softmax/top-2/variance are permutation-invariant).

Capacity: 1280 slots (measured seed-0 max per-expert load is 1091).
Overflow tokens would be dropped gracefully (never corrupt memory).
"""

from contextlib import ExitStack

import numpy as np

import concourse.bass as bass
import concourse.mybir as mybir
import concourse.tile as tile
from concourse import bacc, bass_utils
from concourse.masks import make_identity

# Problem shapes (hardcoded per contract)
B, S, D, F, E, TOPK = 2, 2048, 1024, 4096, 8, 2
T = B * S              # 4096 tokens
N_CORES = 8
CAP = 1280             # gathered-token capacity per expert (bank-aligned splits)
NS = CAP // 128        # 10 slot subblocks
NSB = T // 128         # 32 router subblocks
DC = D // 128          # 8 contraction chunks over D
NFJ = 8                # F stream chunks
FJ = F // NFJ          # 512
NFS = FJ // 128        # 4 F subblocks per chunk
MV_SLICES = [(0, 512), (512, 512), (1024, 256)]  # PSUM-bank-aligned moving splits
DH = 2                 # D output halves of 512
BIG = float(2 ** 20)

f32 = mybir.dt.float32
f32r = mybir.dt.float32r
i32 = mybir.dt.int32
AX = mybir.AxisListType.X
OP = mybir.AluOpType
ACT = mybir.ActivationFunctionType

# cv^2 = var(counts, ddof=1) / (T*K)^2 / (mean_util + 1e-6)^2, mean_util = 1/8
CV_SCALE = float((1.0 / 7.0) / (8192.0 * (0.125 + 1e-6)) ** 2)

_CACHE = {}
LAST_RESULT = None


def _build():
    nc = bacc.Bacc(
        "TRN2", target_bir_lowering=False, debug=False,
        enable_asserts=True, num_devices=N_CORES,
    )
    xin = nc.dram_tensor("xin", [T, D], f32, kind="ExternalInput").ap()
    w1in = nc.dram_tensor("w1in", [D, F], f32, kind="ExternalInput").ap()
    w2in = nc.dram_tensor("w2in", [F, D], f32, kind="ExternalInput").ap()
    b1in = nc.dram_tensor("b1in", [F], f32, kind="ExternalInput").ap()
    b2rep_in = nc.dram_tensor("b2rep", [128, D], f32, kind="ExternalInput").ap()
    wrin = nc.dram_tensor("wrin", [D, E], f32, kind="ExternalInput").ap()
    brrow_in = nc.dram_tensor("brrow", [1, E], f32, kind="ExternalInput").ap()
    iota_in = nc.dram_tensor("iota", [128, NSB], f32, kind="ExternalInput").ap()
    triu_in = nc.dram_tensor("triu", [128, 128], f32, kind="ExternalInput").ap()
    outsh = nc.dram_tensor("outsh", [T // N_CORES, D], f32, kind="ExternalOutput").ap()
    cvout = nc.dram_tensor("cvout", [1, 1], f32, kind="ExternalOutput").ap()

    with tile.TileContext(nc) as tc:
        with (
            tc.tile_pool(name="cpool", bufs=1) as cpool,
            tc.tile_pool(name="gpool", bufs=1) as gpool,
            tc.tile_pool(name="hpool", bufs=1) as hpool,
            tc.tile_pool(name="w1s", bufs=2) as w1s,
            tc.tile_pool(name="w2s", bufs=2) as w2s,
            tc.tile_pool(name="rp", bufs=3) as rp,
            tc.tile_pool(name="sp", bufs=3) as sp,
            tc.tile_pool(name="dram", bufs=1, space="DRAM") as dram,
        ):
            # ---------- constants ----------
            b1t = cpool.tile([128, F // 128], f32)
            nc.sync.dma_start(b1t[:], b1in.rearrange("(fc p) -> p fc", p=128))
            b2rep = cpool.tile([128, D], f32)
            nc.sync.dma_start(b2rep[:], b2rep_in[:])
            wrsb = cpool.tile([128, DC, E], f32)
            nc.sync.dma_start(wrsb[:], wrin.rearrange("(dc p) e -> p dc e", p=128))
            brrow = cpool.tile([1, E], f32)
            nc.sync.dma_start(brrow[:], brrow_in[:])
            iota_sb = cpool.tile([128, NSB], f32)
            nc.sync.dma_start(iota_sb[:], iota_in[:])
            triu = cpool.tile([128, 128], f32)
            nc.sync.dma_start(triu[:], triu_in[:])
            ident = cpool.tile([128, 128], f32)
            make_identity(nc, ident[:])
            ones128 = cpool.tile([128, 1], f32)
            nc.vector.memset(ones128[:], 1.0)
            onesrow = cpool.tile([1, 128], f32)
            nc.vector.memset(onesrow[:], 1.0)
            zeros_sb = cpool.tile([128, D], f32)
            nc.vector.memset(zeros_sb[:], 0.0)
            huge_sb = cpool.tile([128, NS * 2], f32)
            nc.vector.memset(huge_sb[:], BIG)

            # ---------- DRAM scratch ----------
            rs_in = dram.tile([T, D], f32, name="rs_in")
            rs_out = dram.tile([T // N_CORES, D], f32, name="rs_out")
            idxwg_dram = dram.tile([CAP, 2], f32, name="idxwg_dram")

            # prefill pad slots out-of-bounds
            nc.sync.dma_start(
                idxwg_dram.rearrange("(ns p) two -> p ns two", p=128),
                huge_sb[:].rearrange("p (ns two) -> p ns two", two=2))

            # ---------- persistent gathered-state ----------
            x_gT = gpool.tile([128, DC, CAP], f32r, name="x_gT")
            y_acc = gpool.tile([128, NS, D], f32, name="y_acc")
            idx_sb = gpool.tile([128, NS], i32, name="idx_sb")

            # ================= Phase R: router =================
            rctx = ExitStack()
            sps = rctx.enter_context(
                tc.tile_pool(name="sps", bufs=6, space="PSUM"))
            lgp = rctx.enter_context(
                tc.tile_pool(name="lgp", bufs=2, space="PSUM"))
            evall = gpool.tile([128, NSB, E], f32, name="evall")
            for j in range(NSB):
                xrows = rp.tile([128, D], f32, tag="xrows")
                nc.sync.dma_start(xrows[:], xin[j * 128:(j + 1) * 128, :])
                xTt = rp.tile([128, DC, 128], f32, tag="xTt")
                for dc in range(DC):
                    tp = sps.tile([128, 128], f32, tag="sps", space="PSUM")
                    nc.tensor.transpose(
                        tp[:], xrows[:, dc * 128:(dc + 1) * 128], ident[:])
                    if dc % 8 in (0, 2, 4, 6, 7):
                        nc.vector.tensor_copy(xTt[:, dc, :], tp[:])
                    else:
                        nc.scalar.copy(xTt[:, dc, :], tp[:])
                lg = lgp.tile([128, E], f32, tag="lg", space="PSUM")
                for dc in range(DC):
                    nc.tensor.matmul(lg[:], xTt[:, dc, :], wrsb[:, dc, :],
                                     start=(dc == 0), stop=False)
                nc.tensor.matmul(lg[:], onesrow[:], brrow[:],
                                 start=False, stop=True)
                # exp(logits) directly: |logits| small, ratios shift-invariant
                nc.scalar.activation(evall[:, j, :], lg[:], ACT.Exp,
                                     bias=0.0, scale=1.0)

            # batched top-2 / combine weights / counts
            m1 = cpool.tile([128, NSB], f32)
            nc.vector.reduce_max(m1[:], evall[:], axis=AX)
            mlt = cpool.tile([128, NSB, E], f32)
            nc.vector.tensor_tensor(
                mlt[:], evall[:], m1[:, :, None].to_broadcast([128, NSB, E]),
                op=OP.is_lt)
            nc.vector.tensor_tensor(mlt[:], mlt[:], evall[:], op=OP.mult)
            m2 = cpool.tile([128, NSB], f32)
            nc.vector.reduce_max(m2[:], mlt[:], axis=AX)
            den = cpool.tile([128, NSB], f32)
            nc.vector.tensor_tensor(den[:], m1[:], m2[:], op=OP.add)
            rw = cpool.tile([128, NSB], f32)
            nc.vector.reciprocal(rw[:], den[:])
            mskall = cpool.tile([128, NSB, E], f32)
            nc.vector.tensor_tensor(
                mskall[:], evall[:], m2[:, :, None].to_broadcast([128, NSB, E]),
                op=OP.is_ge)
            acc8 = cpool.tile([128, E], f32)
            nc.vector.reduce_sum(
                acc8[:], mskall[:].rearrange("p j e -> p e j"), axis=AX)
            wall = cpool.tile([128, NSB], f32)
            nc.vector.tensor_tensor(wall[:], evall[:, :, 0], mskall[:, :, 0],
                                    op=OP.mult)
            nc.vector.tensor_tensor(wall[:], wall[:], rw[:], op=OP.mult)
            sel0d = cpool.tile([128, NSB], f32)
            nc.vector.tensor_copy(sel0d[:], mskall[:, :, 0])

            # hierarchical exclusive prefix sum over selected tokens
            totp = sps.tile([NSB, 1], f32, tag="sps", space="PSUM")
            nc.tensor.matmul(totp[:], sel0d[:], ones128[:], start=True, stop=True)
            tots = cpool.tile([NSB, 1], f32)
            nc.vector.tensor_copy(tots[:], totp[:])
            offp = sps.tile([1, NSB], f32, tag="sps", space="PSUM")
            nc.tensor.matmul(offp[:], tots[:], triu[0:NSB, 0:NSB],
                             start=True, stop=True)
            offrow = cpool.tile([1, NSB], f32)
            nc.vector.tensor_copy(offrow[:], offp[:])
            pfxp = sps.tile([128, NSB], f32, tag="sps", space="PSUM")
            nc.tensor.matmul(pfxp[:], triu[:], sel0d[:], start=True, stop=False)
            nc.tensor.matmul(pfxp[:], onesrow[:], offrow[:], start=False, stop=True)
            sma = cpool.tile([128, NSB], f32)
            nc.vector.tensor_scalar(sma[:], pfxp[:], BIG, None, op0=OP.subtract)
            nc.vector.tensor_tensor(sma[:], sma[:], sel0d[:], op=OP.mult)
            nc.vector.tensor_scalar(sma[:], sma[:], BIG, None, op0=OP.add)
            s_int = cpool.tile([128, NSB], i32)
            nc.vector.tensor_copy(s_int[:], sma[:])
            pkall = cpool.tile([128, NSB, 2], f32)
            nc.vector.tensor_copy(pkall[:, :, 0], iota_sb[:])
            nc.vector.tensor_copy(pkall[:, :, 1], wall[:])
            for j in range(NSB):
                nc.gpsimd.indirect_dma_start(
                    out=idxwg_dram[:],
                    out_offset=bass.IndirectOffsetOnAxis(
                        ap=s_int[:, j:j + 1], axis=0),
                    in_=pkall[:, j, :],
                    in_offset=None,
                    bounds_check=CAP - 1,
                    oob_is_err=False,
                )

            # cv^2 from counts
            cnt = sps.tile([1, E], f32, tag="sps", space="PSUM")
            nc.tensor.matmul(cnt[:], ones128[:], acc8[:], start=True, stop=True)
            csb = sp.tile([1, E], f32, tag="csb")
            nc.vector.tensor_copy(csb[:], cnt[:])
            mn = sp.tile([1, 1], f32, tag="mn")
            nc.vector.reduce_sum(mn[:], csb[:], axis=AX)
            nc.vector.tensor_scalar_mul(mn[:], mn[:], 0.125)
            dif = sp.tile([1, E], f32, tag="dif")
            nc.vector.tensor_tensor(dif[:], csb[:], mn[:].to_broadcast([1, E]),
                                    op=OP.subtract)
            nc.vector.tensor_tensor(dif[:], dif[:], dif[:], op=OP.mult)
            cv = sp.tile([1, 1], f32, tag="cv")
            nc.vector.reduce_sum(cv[:], dif[:], axis=AX)
            nc.vector.tensor_scalar_mul(cv[:], cv[:], CV_SCALE)
            nc.sync.dma_start(cvout[:], cv[:])

            # ================= Phase G: gather =================
            idxwg_sb = gpool.tile([128, NS, 2], f32, name="idxwg_sb")
            nc.sync.dma_start(
                idxwg_sb[:],
                idxwg_dram.rearrange("(ns p) two -> p ns two", p=128))
            nc.vector.tensor_copy(idx_sb[:], idxwg_sb[:, :, 0])
            for ns in range(NS):
                xg = rp.tile([128, D], f32, tag="xrows", name="xg")
                if ns >= NS - 3:
                    nc.vector.memset(xg[:], 0.0)
                nc.gpsimd.indirect_dma_start(
                    out=xg[:],
                    out_offset=None,
                    in_=xin[:],
                    in_offset=bass.IndirectOffsetOnAxis(
                        ap=idx_sb[:, ns:ns + 1], axis=0),
                    bounds_check=T - 1,
                    oob_is_err=False,
                )
                for dc in range(DC):
                    tp = sps.tile([128, 128], f32, tag="sps", space="PSUM")
                    nc.tensor.transpose(
                        tp[:], xg[:, dc * 128:(dc + 1) * 128], ident[:])
                    if dc % 2 == 0:
                        nc.vector.tensor_copy(
                            x_gT[:, dc, ns * 128:(ns + 1) * 128], tp[:])
                    else:
                        nc.scalar.copy(
                            x_gT[:, dc, ns * 128:(ns + 1) * 128], tp[:])

            # zero-fill the RS input (hidden under phase M, Pool engine queues)
            for j in range(NSB):
                nc.gpsimd.dma_start(rs_in[j * 128:(j + 1) * 128, :], zeros_sb[:])

            # ================= Phase M: MLP over gathered tokens =============
            rctx.close()
            mctx = ExitStack()
            hps_p = mctx.enter_context(
                tc.tile_pool(name="hps_p", bufs=1, space="PSUM"))
            yps_p = mctx.enter_context(
                tc.tile_pool(name="yps_p", bufs=2, space="PSUM"))
            for fj in range(NFJ):
                fj0 = fj * FJ
                w1t = w1s.tile([128, DC, FJ], f32r, tag="w1t")
                nc.sync.dma_start(
                    w1t[:],
                    w1in[:, fj0:fj0 + FJ].rearrange(
                        "(dc p) f -> p dc f", p=128).bitcast(f32r))
                w2t = w2s.tile([128, NFS, D], f32r, tag="w2t")
                nc.sync.dma_start(
                    w2t[:],
                    w2in[fj0:fj0 + FJ, :].rearrange(
                        "(fs p) d -> p fs d", p=128).bitcast(f32r))
                hTc = hpool.tile([128, NFS, CAP], f32r, tag="hTc")
                for fs in range(NFS):
                    hps = hps_p.tile([128, CAP], f32, tag="hps", space="PSUM")
                    for dc in range(DC):
                        for (mv0, mvn) in MV_SLICES:
                            nc.tensor.matmul(
                                hps[:, mv0:mv0 + mvn],
                                w1t[:, dc, fs * 128:(fs + 1) * 128],
                                x_gT[:, dc, mv0:mv0 + mvn],
                                start=(dc == 0), stop=(dc == DC - 1),
                                skip_group_check=True,
                            )
                    nc.scalar.activation(hTc[:, fs, :], hps[:], ACT.Gelu,
                                         bias=b1t[:, fj * NFS + fs: fj * NFS + fs + 1],
                                         scale=1.0)
                for ns in range(NS):
                    nsl = slice(ns * 128, (ns + 1) * 128)
                    yp = yps_p.tile([128, D], f32, tag="yp", space="PSUM",
                                    name=f"yp{fj}_{ns}")
                    for fs in range(NFS):
                        for dh in range(DH):
                            nc.tensor.matmul(
                                yp[:, dh * 512:(dh + 1) * 512],
                                hTc[:, fs, nsl],
                                w2t[:, fs, dh * 512:(dh + 1) * 512],
                                start=(fs == 0), stop=(fs == NFS - 1),
                                skip_group_check=True,
                            )
                    if fj == 0:
                        nc.vector.tensor_copy(y_acc[:, ns, :], yp[:])
                    else:
                        nc.vector.tensor_tensor(
                            y_acc[:, ns, :], y_acc[:, ns, :], yp[:], op=OP.add)

            # ---------- finalize: +b2, *w, scatter to token order ----------
            mctx.close()
            for ns in range(NS):
                outp = rp.tile([128, D], f32, tag="xrows", name="outp")
                nc.vector.tensor_tensor(outp[:], y_acc[:, ns, :], b2rep[:],
                                        op=OP.add)
                nc.vector.tensor_scalar_mul(outp[:], outp[:],
                                            idxwg_sb[:, ns, 1:2])
                nc.gpsimd.indirect_dma_start(
                    out=rs_in[:],
                    out_offset=bass.IndirectOffsetOnAxis(
                        ap=idx_sb[:, ns:ns + 1], axis=0),
                    in_=outp[:],
                    in_offset=None,
                    bounds_check=T - 1,
                    oob_is_err=False,
                )

            # ---------- ReduceScatter over the 8 cores ----------
            nc.gpsimd.collective_compute(
                "ReduceScatter",
                OP.add,
                ins=[rs_in.opt()],
                outs=[rs_out.opt()],
                replica_groups=[list(range(N_CORES))],
            )
            q = (T // N_CORES) // 4
            for k in range(4):
                ob = rp.tile([128, D], f32, tag="xrows", name=f"ob{k}")
                nc.sync.dma_start(ob[:], rs_out[k * q:(k + 1) * q, :])
                nc.sync.dma_start(outsh[k * q:(k + 1) * q, :], ob[:])

    nc.compile()
    return nc


def _get_nc():
    if "nc" not in _CACHE:
        _CACHE["nc"] = _build()
    return _CACHE["nc"]


def kernel(**inputs):
    global LAST_RESULT
    x = np.ascontiguousarray(np.asarray(inputs["x"], dtype=np.float32).reshape(T, D))
    Wr = np.asarray(inputs["Wr"], dtype=np.float32)
    br = np.asarray(inputs["br"], dtype=np.float32)
    W1 = np.asarray(inputs["W1"], dtype=np.float32)
    b1 = np.asarray(inputs["b1"], dtype=np.float32)
    W2 = np.asarray(inputs["W2"], dtype=np.float32)
    b2 = np.asarray(inputs["b2"], dtype=np.float32)

    nc = _get_nc()
    iota = np.ascontiguousarray(
        np.arange(T, dtype=np.float32).reshape(NSB, 128).T)
    triu = np.ascontiguousarray(
        np.triu(np.ones((128, 128), dtype=np.float32), 1))
    in_maps = []
    for c in range(N_CORES):
        e = c
        in_maps.append({
            "xin": x,
            "w1in": np.ascontiguousarray(W1[e]),
            "w2in": np.ascontiguousarray(W2[e]),
            "b1in": np.ascontiguousarray(b1[e]),
            "b2rep": np.ascontiguousarray(
                np.broadcast_to(b2[e][None, :], (128, D))),
            "wrin": np.ascontiguousarray(np.roll(Wr, -e, axis=1)),
            "brrow": np.ascontiguousarray(np.roll(br, -e)[None, :]),
            "iota": iota,
            "triu": triu,
        })
    res = bass_utils.run_bass_kernel_spmd(
        nc, in_maps, core_ids=list(range(N_CORES)))
    LAST_RESULT = res

    out = np.concatenate([res.results[r]["outsh"] for r in range(N_CORES)], axis=0)
    cv2 = np.float32(res.results[0]["cvout"][0, 0])
    return out.reshape(B, S, D), cv2


# revision 28
# speedup vs baseline: 1.0551x; 1.0551x over previous
"""Trainium2 Bass kernel for nn_MoELayer_90202903150800 (MoE, 8 experts, top-2).

Sharding (8 NeuronCores): expert-parallel, one full expert per core, with
SPARSE routing computed on-device:

  Phase R (router): stream x, PE-transpose 128x128 chunks, plain-fp32 logits
    (exact top-2 selection), softmax-free renormalized combine weights
    (w = e_i / (m1 + m2) over top-2 exponentials).  A strict-triangular-matmul
    prefix sum compacts the tokens routed to this core's expert: each selected
    token's slot index s is scattered (indirect DMA) into idx[s] = token id,
    w[s] = combine weight.  Unselected tokens scatter out-of-bounds and are
    dropped.  Top-2 membership counts accumulate for the cv^2 aux loss.
  Phase G (gather): indirect-DMA gather the <=1152 routed token rows of x,
    PE-transpose into x_gT for matmul use.
  Phase M (MLP): stream W1/W2 in 8 F-chunks of 512 (weights pass over SBUF
    exactly once), computing h = gelu(x_g @ W1 + b1) and accumulating
    y = h @ W2 in SBUF, all in fp32r (full-rate fp32 on the PE).
  Finalize: out_partial[token] = (y + b2) * w, scattered back to token order
    in a zero-initialized [4096, 1024] buffer; single ReduceScatter over the
    8 cores sums the 8 expert partials and shards tokens 8 ways; host
    concatenates the shards.  cv^2 is computed on-device from counts.

SPMD: the same program runs on all cores; per-core behaviour differs only
via in_maps data (own expert's weights; Wr columns rolled so the own expert
is column 0 — softmax/top-2/variance are permutation-invariant).

Capacity: 1280 slots (measured seed-0 max per-expert load is 1091).
Overflow tokens would be dropped gracefully (never corrupt memory).
"""

from contextlib import ExitStack

import numpy as np

import concourse.bass as bass
import concourse.mybir as mybir
import concourse.tile as tile
from concourse import bacc, bass_utils
from concourse.masks import make_identity

# Problem shapes (hardcoded per contract)
B, S, D, F, E, TOPK = 2, 2048, 1024, 4096, 8, 2
T = B * S              # 4096 tokens
N_CORES = 8
CAP = 1280             # gathered-token capacity per expert (bank-aligned splits)
NS = CAP // 128        # 10 slot subblocks
NSB = T // 128         # 32 router subblocks
DC = D // 128          # 8 contraction chunks over D
NFJ = 8                # F stream chunks
FJ = F // NFJ          # 512
NFS = FJ // 128        # 4 F subblocks per chunk
MV_SLICES = [(0, 512), (512, 512), (1024, 256)]  # PSUM-bank-aligned moving splits
DH = 2                 # D output halves of 512
BIG = float(2 ** 20)

f32 = mybir.dt.float32
f32r = mybir.dt.float32r
i32 = mybir.dt.int32
AX = mybir.AxisListType.X
OP = mybir.AluOpType
ACT = mybir.ActivationFunctionType

# cv^2 = var(counts, ddof=1) / (T*K)^2 / (mean_util + 1e-6)^2, mean_util = 1/8
CV_SCALE = float((1.0 / 7.0) / (8192.0 * (0.125 + 1e-6)) ** 2)

_CACHE = {}
LAST_RESULT = None


def _build():
    nc = bacc.Bacc(
        "TRN2", target_bir_lowering=False, debug=False,
        enable_asserts=True, num_devices=N_CORES,
    )
    xin = nc.dram_tensor("xin", [T, D], f32, kind="ExternalInput").ap()
    w1in = nc.dram_tensor("w1in", [D, F], f32, kind="ExternalInput").ap()
    w2in = nc.dram_tensor("w2in", [F, D], f32, kind="ExternalInput").ap()
    b1in = nc.dram_tensor("b1in", [F], f32, kind="ExternalInput").ap()
    b2rep_in = nc.dram_tensor("b2rep", [128, D], f32, kind="ExternalInput").ap()
    wrin = nc.dram_tensor("wrin", [D, E], f32, kind="ExternalInput").ap()
    brrow_in = nc.dram_tensor("brrow", [1, E], f32, kind="ExternalInput").ap()
    iota_in = nc.dram_tensor("iota", [128, NSB], f32, kind="ExternalInput").ap()
    triu_in = nc.dram_tensor("triu", [128, 128], f32, kind="ExternalInput").ap()
    outsh = nc.dram_tensor("outsh", [T // N_CORES, D], f32, kind="ExternalOutput").ap()
    cvout = nc.dram_tensor("cvout", [1, 1], f32, kind="ExternalOutput").ap()

    with tile.TileContext(nc) as tc:
        with (
            tc.tile_pool(name="cpool", bufs=1) as cpool,
            tc.tile_pool(name="gpool", bufs=1) as gpool,
            tc.tile_pool(name="hpool", bufs=1) as hpool,
            tc.tile_pool(name="w1s", bufs=2) as w1s,
            tc.tile_pool(name="w2s", bufs=1) as w2s,
            tc.tile_pool(name="rp", bufs=4) as rp,
            tc.tile_pool(name="sp", bufs=3) as sp,
            tc.tile_pool(name="dram", bufs=1, space="DRAM") as dram,
        ):
            # ---------- constants ----------
            b1t = cpool.tile([128, F // 128], f32)
            nc.sync.dma_start(b1t[:], b1in.rearrange("(fc p) -> p fc", p=128))
            b2rep = cpool.tile([128, D], f32)
            nc.sync.dma_start(b2rep[:], b2rep_in[:])
            wrsb = cpool.tile([128, DC, E], f32)
            nc.sync.dma_start(wrsb[:], wrin.rearrange("(dc p) e -> p dc e", p=128))
            brrow = cpool.tile([1, E], f32)
            nc.sync.dma_start(brrow[:], brrow_in[:])
            iota_sb = cpool.tile([128, NSB], f32)
            nc.sync.dma_start(iota_sb[:], iota_in[:])
            triu = cpool.tile([128, 128], f32)
            nc.sync.dma_start(triu[:], triu_in[:])
            ident = cpool.tile([128, 128], f32)
            make_identity(nc, ident[:])
            ones128 = cpool.tile([128, 1], f32)
            nc.vector.memset(ones128[:], 1.0)
            onesrow = cpool.tile([1, 128], f32)
            nc.vector.memset(onesrow[:], 1.0)
            zeros_sb = cpool.tile([128, D], f32)
            nc.vector.memset(zeros_sb[:], 0.0)
            huge_sb = cpool.tile([128, NS * 2], f32)
            nc.vector.memset(huge_sb[:], BIG)

            # ---------- DRAM scratch ----------
            rs_in = dram.tile([T, D], f32, name="rs_in")
            rs_out = dram.tile([T // N_CORES, D], f32, name="rs_out")
            idxwg_dram = dram.tile([CAP, 2], f32, name="idxwg_dram")

            # prefill pad slots out-of-bounds
            nc.sync.dma_start(
                idxwg_dram.rearrange("(ns p) two -> p ns two", p=128),
                huge_sb[:].rearrange("p (ns two) -> p ns two", two=2))

            # ---------- persistent gathered-state ----------
            x_gT = gpool.tile([128, DC, CAP], f32r, name="x_gT")
            y_acc = gpool.tile([128, NS, D], f32, name="y_acc")
            idx_sb = gpool.tile([128, NS], i32, name="idx_sb")

            # ================= Phase R: router =================
            rctx = ExitStack()
            sps = rctx.enter_context(
                tc.tile_pool(name="sps", bufs=6, space="PSUM"))
            lgp = rctx.enter_context(
                tc.tile_pool(name="lgp", bufs=2, space="PSUM"))
            evall = gpool.tile([128, NSB, E], f32, name="evall")
            for j in range(NSB):
                xrows = rp.tile([128, D], f32, tag="xrows")
                nc.sync.dma_start(xrows[:], xin[j * 128:(j + 1) * 128, :])
                xTt = rp.tile([128, DC, 128], f32, tag="xTt")
                for dc in range(DC):
                    tp = sps.tile([128, 128], f32, tag="sps", space="PSUM")
                    nc.tensor.transpose(
                        tp[:], xrows[:, dc * 128:(dc + 1) * 128], ident[:])
                    if dc % 8 in (0, 2, 4, 6, 7):
                        nc.vector.tensor_copy(xTt[:, dc, :], tp[:])
                    else:
                        nc.scalar.copy(xTt[:, dc, :], tp[:])
                lg = lgp.tile([128, E], f32, tag="lg", space="PSUM")
                for dc in range(DC):
                    nc.tensor.matmul(lg[:], xTt[:, dc, :], wrsb[:, dc, :],
                                     start=(dc == 0), stop=False)
                nc.tensor.matmul(lg[:], onesrow[:], brrow[:],
                                 start=False, stop=True)
                # exp(logits) directly: |logits| small, ratios shift-invariant
                nc.scalar.activation(evall[:, j, :], lg[:], ACT.Exp,
                                     bias=0.0, scale=1.0)

            # batched top-2 / combine weights / counts
            m1 = cpool.tile([128, NSB], f32)
            nc.vector.reduce_max(m1[:], evall[:], axis=AX)
            mlt = cpool.tile([128, NSB, E], f32)
            nc.vector.tensor_tensor(
                mlt[:], evall[:], m1[:, :, None].to_broadcast([128, NSB, E]),
                op=OP.is_lt)
            nc.vector.tensor_tensor(mlt[:], mlt[:], evall[:], op=OP.mult)
            m2 = cpool.tile([128, NSB], f32)
            nc.vector.reduce_max(m2[:], mlt[:], axis=AX)
            den = cpool.tile([128, NSB], f32)
            nc.vector.tensor_tensor(den[:], m1[:], m2[:], op=OP.add)
            rw = cpool.tile([128, NSB], f32)
            nc.vector.reciprocal(rw[:], den[:])
            mskall = cpool.tile([128, NSB, E], f32)
            nc.vector.tensor_tensor(
                mskall[:], evall[:], m2[:, :, None].to_broadcast([128, NSB, E]),
                op=OP.is_ge)
            acc8 = cpool.tile([128, E], f32)
            nc.vector.reduce_sum(
                acc8[:], mskall[:].rearrange("p j e -> p e j"), axis=AX)
            wall = cpool.tile([128, NSB], f32)
            nc.vector.tensor_tensor(wall[:], evall[:, :, 0], mskall[:, :, 0],
                                    op=OP.mult)
            nc.vector.tensor_tensor(wall[:], wall[:], rw[:], op=OP.mult)
            sel0d = cpool.tile([128, NSB], f32)
            nc.vector.tensor_copy(sel0d[:], mskall[:, :, 0])

            # hierarchical exclusive prefix sum over selected tokens
            totp = sps.tile([NSB, 1], f32, tag="sps", space="PSUM")
            nc.tensor.matmul(totp[:], sel0d[:], ones128[:], start=True, stop=True)
            tots = cpool.tile([NSB, 1], f32)
            nc.vector.tensor_copy(tots[:], totp[:])
            offp = sps.tile([1, NSB], f32, tag="sps", space="PSUM")
            nc.tensor.matmul(offp[:], tots[:], triu[0:NSB, 0:NSB],
                             start=True, stop=True)
            offrow = cpool.tile([1, NSB], f32)
            nc.vector.tensor_copy(offrow[:], offp[:])
            pfxp = sps.tile([128, NSB], f32, tag="sps", space="PSUM")
            nc.tensor.matmul(pfxp[:], triu[:], sel0d[:], start=True, stop=False)
            nc.tensor.matmul(pfxp[:], onesrow[:], offrow[:], start=False, stop=True)
            sma = cpool.tile([128, NSB], f32)
            nc.vector.tensor_scalar(sma[:], pfxp[:], BIG, None, op0=OP.subtract)
            nc.vector.tensor_tensor(sma[:], sma[:], sel0d[:], op=OP.mult)
            nc.vector.tensor_scalar(sma[:], sma[:], BIG, None, op0=OP.add)
            s_int = cpool.tile([128, NSB], i32)
            nc.vector.tensor_copy(s_int[:], sma[:])
            pkall = cpool.tile([128, NSB, 2], f32)
            nc.vector.tensor_copy(pkall[:, :, 0], iota_sb[:])
            nc.vector.tensor_copy(pkall[:, :, 1], wall[:])
            for j in range(NSB):
                nc.gpsimd.indirect_dma_start(
                    out=idxwg_dram[:],
                    out_offset=bass.IndirectOffsetOnAxis(
                        ap=s_int[:, j:j + 1], axis=0),
                    in_=pkall[:, j, :],
                    in_offset=None,
                    bounds_check=CAP - 1,
                    oob_is_err=False,
                )

            # cv^2 from counts
            cnt = sps.tile([1, E], f32, tag="sps", space="PSUM")
            nc.tensor.matmul(cnt[:], ones128[:], acc8[:], start=True, stop=True)
            csb = sp.tile([1, E], f32, tag="csb")
            nc.vector.tensor_copy(csb[:], cnt[:])
            mn = sp.tile([1, 1], f32, tag="mn")
            nc.vector.reduce_sum(mn[:], csb[:], axis=AX)
            nc.vector.tensor_scalar_mul(mn[:], mn[:], 0.125)
            dif = sp.tile([1, E], f32, tag="dif")
            nc.vector.tensor_tensor(dif[:], csb[:], mn[:].to_broadcast([1, E]),
                                    op=OP.subtract)
            nc.vector.tensor_tensor(dif[:], dif[:], dif[:], op=OP.mult)
            cv = sp.tile([1, 1], f32, tag="cv")
            nc.vector.reduce_sum(cv[:], dif[:], axis=AX)
            nc.vector.tensor_scalar_mul(cv[:], cv[:], CV_SCALE)
            nc.sync.dma_start(cvout[:], cv[:])

            # ================= Phase G: gather =================
            idxwg_sb = gpool.tile([128, NS, 2], f32, name="idxwg_sb")
            nc.sync.dma_start(
                idxwg_sb[:],
                idxwg_dram.rearrange("(ns p) two -> p ns two", p=128))
            nc.vector.tensor_copy(idx_sb[:], idxwg_sb[:, :, 0])
            for ns in range(NS):
                xg = rp.tile([128, D], f32, tag="xrows", name="xg")
                if ns >= NS - 3:
                    nc.vector.memset(xg[:], 0.0)
                nc.gpsimd.indirect_dma_start(
                    out=xg[:],
                    out_offset=None,
                    in_=xin[:],
                    in_offset=bass.IndirectOffsetOnAxis(
                        ap=idx_sb[:, ns:ns + 1], axis=0),
                    bounds_check=T - 1,
                    oob_is_err=False,
                )
                for dc in range(DC):
                    tp = sps.tile([128, 128], f32, tag="sps", space="PSUM")
                    nc.tensor.transpose(
                        tp[:], xg[:, dc * 128:(dc + 1) * 128], ident[:])
                    if dc % 2 == 0:
                        nc.vector.tensor_copy(
                            x_gT[:, dc, ns * 128:(ns + 1) * 128], tp[:])
                    else:
                        nc.scalar.copy(
                            x_gT[:, dc, ns * 128:(ns + 1) * 128], tp[:])

            # zero-fill the RS input (hidden under phase M, Pool engine queues)
            for j in range(NSB):
                nc.gpsimd.dma_start(rs_in[j * 128:(j + 1) * 128, :], zeros_sb[:])

            # ================= Phase M: MLP over gathered tokens =============
            rctx.close()
            mctx = ExitStack()
            hps_p = mctx.enter_context(
                tc.tile_pool(name="hps_p", bufs=1, space="PSUM"))
            yps_p = mctx.enter_context(
                tc.tile_pool(name="yps_p", bufs=2, space="PSUM"))
            for fj in range(NFJ):
                fj0 = fj * FJ
                w1t = w1s.tile([128, DC, FJ], f32r, tag="w1t")
                nc.sync.dma_start(
                    w1t[:],
                    w1in[:, fj0:fj0 + FJ].rearrange(
                        "(dc p) f -> p dc f", p=128).bitcast(f32r))
                w2t = w2s.tile([128, NFS, D], f32r, tag="w2t")
                nc.sync.dma_start(
                    w2t[:],
                    w2in[fj0:fj0 + FJ, :].rearrange(
                        "(fs p) d -> p fs d", p=128).bitcast(f32r))
                hTc = hpool.tile([128, NFS, CAP], f32r, tag="hTc")
                for fs in range(NFS):
                    hps = hps_p.tile([128, CAP], f32, tag="hps", space="PSUM")
                    for dc in range(DC):
                        for (mv0, mvn) in MV_SLICES:
                            nc.tensor.matmul(
                                hps[:, mv0:mv0 + mvn],
                                w1t[:, dc, fs * 128:(fs + 1) * 128],
                                x_gT[:, dc, mv0:mv0 + mvn],
                                start=(dc == 0), stop=(dc == DC - 1),
                                skip_group_check=True,
                            )
                    nc.scalar.activation(hTc[:, fs, :], hps[:], ACT.Gelu,
                                         bias=b1t[:, fj * NFS + fs: fj * NFS + fs + 1],
                                         scale=1.0)
                for ns in range(NS):
                    nsl = slice(ns * 128, (ns + 1) * 128)
                    yp = yps_p.tile([128, D], f32, tag="yp", space="PSUM",
                                    name=f"yp{fj}_{ns}")
                    for fs in range(NFS):
                        for dh in range(DH):
                            nc.tensor.matmul(
                                yp[:, dh * 512:(dh + 1) * 512],
                                hTc[:, fs, nsl],
                                w2t[:, fs, dh * 512:(dh + 1) * 512],
                                start=(fs == 0), stop=(fs == NFS - 1),
                                skip_group_check=True,
                            )
                    if fj == 0:
                        nc.vector.tensor_copy(y_acc[:, ns, :], yp[:])
                    elif fj < NFJ - 1:
                        nc.vector.tensor_tensor(
                            y_acc[:, ns, :], y_acc[:, ns, :], yp[:], op=OP.add)
                    else:
                        # fused finalize: (y + b2) * w, scatter to token order
                        outp = rp.tile([128, D], f32, tag="xrows",
                                       name=f"outp{ns}")
                        nc.vector.tensor_tensor(outp[:], y_acc[:, ns, :], yp[:],
                                                op=OP.add)
                        nc.vector.tensor_tensor(outp[:], outp[:], b2rep[:],
                                                op=OP.add)
                        nc.vector.tensor_scalar_mul(outp[:], outp[:],
                                                    idxwg_sb[:, ns, 1:2])
                        nc.gpsimd.indirect_dma_start(
                            out=rs_in[:],
                            out_offset=bass.IndirectOffsetOnAxis(
                                ap=idx_sb[:, ns:ns + 1], axis=0),
                            in_=outp[:],
                            in_offset=None,
                            bounds_check=T - 1,
                            oob_is_err=False,
                        )

            mctx.close()

            # ---------- ReduceScatter over the 8 cores ----------
            nc.gpsimd.collective_compute(
                "ReduceScatter",
                OP.add,
                ins=[rs_in.opt()],
                outs=[rs_out.opt()],
                replica_groups=[list(range(N_CORES))],
            )
            q = (T // N_CORES) // 4
            for k in range(4):
                ob = rp.tile([128, D], f32, tag="xrows", name=f"ob{k}")
                nc.sync.dma_start(ob[:], rs_out[k * q:(k + 1) * q, :])
                nc.sync.dma_start(outsh[k * q:(k + 1) * q, :], ob[:])

    nc.compile()
    return nc


def _get_nc():
    if "nc" not in _CACHE:
        _CACHE["nc"] = _build()
    return _CACHE["nc"]


def kernel(**inputs):
    global LAST_RESULT
    x = np.ascontiguousarray(np.asarray(inputs["x"], dtype=np.float32).reshape(T, D))
    Wr = np.asarray(inputs["Wr"], dtype=np.float32)
    br = np.asarray(inputs["br"], dtype=np.float32)
    W1 = np.asarray(inputs["W1"], dtype=np.float32)
    b1 = np.asarray(inputs["b1"], dtype=np.float32)
    W2 = np.asarray(inputs["W2"], dtype=np.float32)
    b2 = np.asarray(inputs["b2"], dtype=np.float32)

    nc = _get_nc()
    iota = np.ascontiguousarray(
        np.arange(T, dtype=np.float32).reshape(NSB, 128).T)
    triu = np.ascontiguousarray(
        np.triu(np.ones((128, 128), dtype=np.float32), 1))
    in_maps = []
    for c in range(N_CORES):
        e = c
        in_maps.append({
            "xin": x,
            "w1in": np.ascontiguousarray(W1[e]),
            "w2in": np.ascontiguousarray(W2[e]),
            "b1in": np.ascontiguousarray(b1[e]),
            "b2rep": np.ascontiguousarray(
                np.broadcast_to(b2[e][None, :], (128, D))),
            "wrin": np.ascontiguousarray(np.roll(Wr, -e, axis=1)),
            "brrow": np.ascontiguousarray(np.roll(br, -e)[None, :]),
            "iota": iota,
            "triu": triu,
        })
    res = bass_utils.run_bass_kernel_spmd(
        nc, in_maps, core_ids=list(range(N_CORES)))
    LAST_RESULT = res

    out = np.concatenate([res.results[r]["outsh"] for r in range(N_CORES)], axis=0)
    cv2 = np.float32(res.results[0]["cvout"][0, 0])
    return out.reshape(B, S, D), cv2
